# revision 25
# baseline (speedup 1.0000x reference)
"""Trainium2 Bass kernel for the MDA GNN (3x GAT views + MS-CAM fusion + pair MLP).

Distribution over 8 NeuronCores, core c = (a, b): a = c % 4 (row quarter),
b = c // 4 (output-feature half; 901 -> 2x452).

Design (vs the AllGather baseline):
  - attention logits asrc/adst are host-precomputed matvecs, shipped as tiny
    inputs: no featU on device, no logit columns in the collective payload;
    asrc[j] is folded into the mask tensor (mask = asrc + {0 | -100})
  - softmax weights pt[j, i] for ALL 1784 fused rows i depend only on
    inputs, so DVE/Act compute them from t=0, overlapped with stage 1:
    em = adst + mask (2x DVE), lrelu via (0.2*em) max em (4x + 2x DVE),
    exp on Act (fp16 out; masked entries underflow fp16 to ~0)
  - stage 1: h[j in quarter a, half b] = feat @ W[half].T in fp16,
    k-tile-outer over j-subtile groups of 4 so PE starts on the first
    featT chunk; fp16 h tiles stay in SBUF
  - stage 2: local-j partial attention sums [1784, 453] (452 cols + rowsum
    via an ones column) in fp16; all three views packed into ONE fp16
    ReduceScatter over each 4-core half-group -- replaces the three big
    AllGathers and the agout round-trips entirely
  - epilogue v = relu(num/rowsum + b), MS-CAM in fp16 on DVE fast modes,
    BN stats per-core (LOCAL_STATS) or via two tiny AllGathers
  - output per-core q,r partials; host sums halves and applies the
    collapsed pair MLP
"""

import numpy as np
import ml_dtypes

import concourse.bass as bass
import concourse.mybir as mybir
import concourse.tile as tile
from concourse import bacc
from concourse.bass_utils import run_bass_kernel_spmd

F16 = mybir.dt.float16
F32 = mybir.dt.float32
AF = mybir.ActivationFunctionType
MUL = mybir.AluOpType.mult
ADD = mybir.AluOpType.add
MAX = mybir.AluOpType.max

NCORES = 8
NA = 4
OUT = 901
OH = 452              # half width (904 = 2*452)
OHR = OH + 1          # + rowsum column
NROWS = 1778
MI = 1784
CI = MI // NA         # 446 rows per core after scatter
NIT = -(-MI // 128)   # stage-2 i tiles (14, last = 120)
NPAIRS = 4096
EPS = 1e-5
CNT = float(NROWS * OUT)
MASKNEG = -100.0
LOCAL_STATS = True   # per-core BN stats (approx) instead of global AGs

VIEWS = [
    dict(name="mrna", N=3929, off=3052),
    dict(name="inc", N=2459, off=1582),
    dict(name="drug", N=2060, off=1183),
]
for V in VIEWS:
    V["CJ"] = -(-V["N"] // NA)               # per-core source rows
    V["NK"] = -(-V["N"] // 128)
    V["KP"] = V["NK"] * 128
    V["NJS"] = -(-V["CJ"] // 128)            # j subtiles

ISUBS = [(0, 128), (128, 128), (256, 128), (384, CI - 384)]

_CACHE = {}
LAST_RESULTS = None


def _bcast(ap, parts, cols, offset=0):
    """Partition-broadcast AP over a DRAM row."""
    return bass.AP(tensor=ap.tensor, offset=ap.offset + offset,
                   ap=[[0, parts], [1, cols]])


def build_graph():
    nc = bacc.Bacc("TRN2", target_bir_lowering=False, debug=False,
                   enable_asserts=False, num_devices=NCORES)
    ins = {}
    for V in VIEWS:
        n = V["name"]
        ins[f"featT_{n}"] = nc.dram_tensor(
            f"featT_{n}", [128, V["NK"] * V["CJ"]], F16, kind="ExternalInput").ap()
        ins[f"Wx_{n}"] = nc.dram_tensor(
            f"Wx_{n}", [128, V["NK"] * OH], F16, kind="ExternalInput").ap()
        ins[f"maskb_{n}"] = nc.dram_tensor(
            f"maskb_{n}", [V["NJS"] * 128, MI], F16, kind="ExternalInput").ap()
        ins[f"adst_{n}"] = nc.dram_tensor(
            f"adst_{n}", [1, MI], F16, kind="ExternalInput").ap()
        ins[f"b_{n}"] = nc.dram_tensor(
            f"b_{n}", [1, OH], F16, kind="ExternalInput").ap()
    ins["md"] = nc.dram_tensor("md", [CI, OH], F16, kind="ExternalInput").ap()
    ins["validi"] = nc.dram_tensor("validi", [CI, 1], F32, kind="ExternalInput").ap()
    ins["camw"] = nc.dram_tensor("camw", [1, 16], F32, kind="ExternalInput").ap()
    ins["wab"] = nc.dram_tensor("wab", [2, OH], F16, kind="ExternalInput").ap()
    ins["cntinv"] = nc.dram_tensor("cntinv", [1, 1], F32, kind="ExternalInput").ap()
    qr_out = nc.dram_tensor("qr", [CI, 2], F32, kind="ExternalOutput").ap()
    rg_half = [[0, 1, 2, 3], [4, 5, 6, 7]]
    rg_all = [list(range(NCORES))]
    NV = len(VIEWS)
    FTMAX = max(V["NK"] * V["CJ"] for V in VIEWS)
    WXMAX = max(V["NK"] * OH for V in VIEWS)

    with tile.TileContext(nc) as tc:
        with (
            tc.tile_pool(name="persist", bufs=1) as per,
            tc.tile_pool(name="stream", bufs=2) as st,
            tc.tile_pool(name="dram", bufs=1, space="DRAM") as dr,
            tc.tile_pool(name="ps_s1", bufs=1, space="PSUM") as ps1,
            tc.tile_pool(name="ps_s2", bufs=2, space="PSUM") as ps2p,
            tc.tile_pool(name="ps_sm", bufs=1, space="PSUM") as pss,
        ):
            # ---- non-DMA constants ----
            ones = per.tile([128, 1], F32, tag="ones")
            nc.vector.memset(ones, 1.0)
            onesrow = per.tile([1, 128], F32, tag="onesrow")
            nc.vector.memset(onesrow, 1.0)
            epst = per.tile([1, 1], F32, tag="epst")
            nc.vector.memset(epst, EPS)
            camb = per.tile([128, 16], F32, tag="camb")
            cnti = per.tile([1, 1], F32, tag="cnti")
            valid, invalid, mdt = {}, {}, {}
            adstbc, bbc = {}, {}

            def late_dmas():
                # everything not needed until the epilogue/CAM tail
                nc.sync.dma_start(camb, _bcast(ins["camw"], 128, 16))
                nc.sync.dma_start(cnti, ins["cntinv"][:, :])
                for s, (i0, isz) in enumerate(ISUBS):
                    valid[s] = per.tile([128, 1], F32, tag=f"valid{s}",
                                        name=f"valid{s}")
                    nc.sync.dma_start(valid[s][:isz], ins["validi"][i0:i0 + isz, :])
                    invalid[s] = per.tile([128, 1], F32, tag=f"invalid{s}",
                                          name=f"invalid{s}")
                    nc.vector.tensor_scalar(invalid[s][:isz], valid[s][:isz],
                                            -1.0, 1.0, op0=MUL, op1=ADD)
                wabc = per.tile([128, 2 * OH], F16, tag="wabc")
                nc.sync.dma_start(wabc[:, 0:OH], _bcast(ins["wab"], 128, OH, offset=0))
                nc.sync.dma_start(wabc[:, OH:2 * OH],
                                  _bcast(ins["wab"], 128, OH, offset=OH))
                for vi, V in enumerate(VIEWS):
                    n = V["name"]
                    t = per.tile([128, OH], F16, tag=f"bbc{vi}", name=f"bbc{vi}")
                    nc.sync.dma_start(t, _bcast(ins[f"b_{n}"], 128, OH))
                    bbc[vi] = t
                return wabc

            # single featT/Wx buffers, reused across views (WAR-tracked)
            ftbuf = per.tile([128, FTMAX], F16, tag="ftbuf")
            wxbuf = per.tile([128, WXMAX], F16, tag="wxbuf")
            mbt = {}

            rsin = {vi: dr.tile([MI, OHR], F16, tag=f"rsin{vi}", name=f"rsin{vi}")
                    for vi in range(NV)}
            rsout = {vi: dr.tile([CI, OHR], F16, tag=f"rsout{vi}", name=f"rsout{vi}")
                     for vi in range(NV)}

            hsub, pts = {}, {}
            for vi, V in enumerate(VIEWS):
                n, CJ, NK, NJS = V["name"], V["CJ"], V["NK"], V["NJS"]
                # -------- input DMAs: first weight chunks lead the queue ----
                t = per.tile([128, MI], F16, tag=f"adstbc{vi}", name=f"adstbc{vi}")
                adstbc[vi] = t
                nch = 4
                ktc = [(NK * c // nch, NK * (c + 1) // nch) for c in range(nch)]
                for ci, (k0, k1) in enumerate(ktc):
                    nc.sync.dma_start(wxbuf[:, k0 * OH:k1 * OH],
                                      ins[f"Wx_{n}"][:, k0 * OH:k1 * OH])
                    nc.sync.dma_start(ftbuf[:, k0 * CJ:k1 * CJ],
                                      ins[f"featT_{n}"][:, k0 * CJ:k1 * CJ])
                    if ci == 0:
                        nc.sync.dma_start(t, _bcast(ins[f"adst_{n}"], 128, MI))
                        for s in range(min(2, NJS)):
                            mb = st.tile([128, MI], F16, tag="mb", bufs=5,
                                         name=f"mb{vi}_{s}")
                            nc.sync.dma_start(
                                mb, ins[f"maskb_{n}"][s * 128:(s + 1) * 128, :])
                            mbt[(vi, s)] = mb
                for s in range(2, NJS):
                    mb = st.tile([128, MI], F16, tag="mb", bufs=5,
                                 name=f"mb{vi}_{s}")
                    nc.sync.dma_start(mb, ins[f"maskb_{n}"][s * 128:(s + 1) * 128, :])
                    mbt[(vi, s)] = mb
                if vi == 0:
                    wabc = late_dmas()

                # -------- stage 1: kt-outer over j-subtile groups of 4 -----
                for s in range(NJS):
                    hsub[(vi, s)] = per.tile([128, OHR], F16, tag=f"h{s}",
                                             name=f"h{vi}_{s}")
                groups = [list(range(g, min(g + 4, NJS))) for g in range(0, NJS, 4)]
                for grp in groups:
                    hp = {s: ps1.tile([128, OH], F32, tag=f"s1ps{gi}",
                                      name=f"s1ps{vi}_{s}")
                          for gi, s in enumerate(grp)}
                    for kt in range(NK):
                        for s in grp:
                            pj = min(128, CJ - s * 128)
                            nc.tensor.matmul(
                                hp[s][:pj],
                                ftbuf[:, kt * CJ + s * 128: kt * CJ + s * 128 + pj],
                                wxbuf[:, kt * OH:(kt + 1) * OH],
                                start=(kt == 0), stop=(kt == NK - 1))
                    for s in grp:
                        pj = min(128, CJ - s * 128)
                        ht = hsub[(vi, s)]
                        if pj < 128:
                            nc.vector.memset(ht, 0.0)
                        nc.scalar.copy(ht[:pj, 0:OH], hp[s][:pj])
                        nc.vector.memset(ht[:pj, OH:OHR], 1.0)

                # -------- softmax weights (DVE/Act, input-only deps) -------
                for s in range(NJS):
                    pts[(vi, s)] = per.tile([128, MI], F16, tag=f"pt{s}",
                                            name=f"pt{vi}_{s}")
                    em = st.tile([128, MI], F16, tag="em", bufs=2)
                    pt = pts[(vi, s)]
                    nc.vector.tensor_add(em, adstbc[vi], mbt[(vi, s)])
                    nc.vector.tensor_scalar_mul(pt, em, 0.2)
                    nc.vector.tensor_max(em, em, pt)
                    nc.scalar.activation(pt, em, AF.Exp)

                # -------- stage 2: partial sums over local j ---------------
                for it in range(NIT):
                    isz = min(128, MI - it * 128)
                    i0 = it * 128
                    pp = ps2p.tile([128, OHR], F32, tag="s2ps")
                    for s in range(NJS):
                        nc.tensor.matmul(
                            pp[:isz], pts[(vi, s)][:, i0:i0 + isz],
                            hsub[(vi, s)][:, :],
                            start=(s == 0), stop=(s == NJS - 1))
                    ct = st.tile([128, OHR], F16, tag="ct", bufs=3)
                    if it % 2 == 0:
                        nc.vector.tensor_copy(ct[:isz], pp[:isz])
                    else:
                        nc.scalar.copy(ct[:isz], pp[:isz])
                    nc.sync.dma_start(rsin[vi][i0:i0 + isz, :], ct[:isz])

                nc.gpsimd.collective_compute(
                    "ReduceScatter", ADD, replica_groups=rg_half,
                    ins=[rsin[vi].opt()], outs=[rsout[vi].opt()])

            # =================== epilogue + CAM (packed) ===============
            # xsp[vi][:, s*OH:(s+1)*OH] = x for subtile s; xsp[3] = mirna_disease
            # channel attention channel c -> (drug, inc, mrna, md) = view
            # indices (2, 1, 0, md); cmap[vi] = channel of view vi
            PW = len(ISUBS) * OH
            CH = [2, 1, 0, 3]
            CMAP = [2, 1, 0]
            xsp = {c: per.tile([128, PW], F16, tag=f"xsp{c}", name=f"xsp{c}")
                   for c in range(4)}
            for c in range(4):
                nc.vector.memset(xsp[c], 0.0)
            for s, (i0, isz) in enumerate(ISUBS):
                nc.sync.dma_start(xsp[3][:isz, s * OH:(s + 1) * OH],
                                  ins["md"][i0:i0 + isz, :])
            # y1 initialized with the input-only md term (channel 3)
            y1 = {}
            tm = st.tile([128, PW], F16, tag="y1tmp", bufs=1)
            for br, coff in (("l", 0), ("g", 4)):
                t = per.tile([128, PW], F16, tag=f"y1{br}", name=f"y1{br}")
                nc.vector.tensor_scalar_mul(t, xsp[3], camb[:, coff + 3:coff + 4])
                y1[br] = t
            for vi in range(NV):
                for s, (i0, isz) in enumerate(ISUBS):
                    rsg = st.tile([128, OHR], F16, tag="rsg", bufs=2)
                    nc.sync.dma_start(rsg[:isz], rsout[vi][i0:i0 + isz, :])
                    rsum = st.tile([128, 1], F32, tag="rsum")
                    nc.vector.tensor_add(rsum[:isz], rsg[:isz, OH:OHR],
                                         invalid[s][:isz])
                    rs = st.tile([128, 1], F32, tag="rs")
                    nc.vector.reciprocal(rs[:isz], rsum[:isz])
                    xv = xsp[vi][:, s * OH:(s + 1) * OH]
                    nc.vector.tensor_scalar_mul(xv[:isz], rsg[:isz, 0:OH], rs[:isz])
                    nc.vector.tensor_add(xv[:isz], xv[:isz], bbc[vi][:isz])
                    nc.vector.tensor_scalar_max(xv[:isz], xv[:isz], 0.0)
                for br, coff in (("l", 0), ("g", 4)):
                    cc = coff + CMAP[vi]
                    nc.vector.tensor_scalar_mul(tm, xsp[vi], camb[:, cc:cc + 1])
                    nc.vector.tensor_add(y1[br], y1[br], tm)

            def pe_bcast(src_row, cols, tag):
                """Broadcast [1, cols] SBUF row to [128, cols] via PE."""
                bps = pss.tile([128, 16], F32, tag="bps", name=f"bps{tag}")[:, 0:cols]
                nc.tensor.matmul(bps, onesrow[0:1, :], src_row,
                                 start=True, stop=True)
                out = per.tile([128, cols], F32, tag=f"bc{tag}", name=f"bc{tag}")
                nc.vector.tensor_copy(out, bps)
                return out

            def stats_round(srcs, tag):
                # per-core sums over valid rows: cols (S_l, S_g, Q_l, Q_g)
                stp = pss.tile([1, 4], F32, tag="small")
                nsub = len(ISUBS)
                for s, (i0, isz) in enumerate(ISUBS):
                    sc = st.tile([128, 4], F32, tag="scst", bufs=2)
                    sq = st.tile([128, OH], F32, tag="sqscr", bufs=1)
                    for bi, br in enumerate(("l", "g")):
                        ysl = srcs[br][:, s * OH:(s + 1) * OH]
                        nc.vector.reduce_sum(sc[:isz, bi:bi + 1], ysl[:isz],
                                             axis=mybir.AxisListType.X)
                        nc.scalar.activation(sq[:isz], ysl[:isz],
                                             AF.Square,
                                             accum_out=sc[:isz, 2 + bi:3 + bi])
                    nc.vector.tensor_scalar_mul(sc[:isz], sc[:isz], valid[s][:isz])
                    nc.tensor.matmul(stp[:1], ones[:isz], sc[:isz],
                                     start=(s == 0), stop=(s == nsub - 1))
                if LOCAL_STATS:
                    gsb = st.tile([1, 4], F32, tag=f"loc{tag}", name=f"loc{tag}")
                    nc.vector.tensor_copy(gsb, stp)
                    mrow = per.tile([1, 4], F32, tag=f"mrow{tag}", name=f"mrow{tag}")
                    nc.vector.tensor_scalar_mul(mrow, gsb, cnti[0:1, 0:1])
                else:
                    loc = st.tile([1, 4], F32, tag=f"loc{tag}", name=f"loc{tag}")
                    nc.vector.tensor_copy(loc, stp)
                    agi = dr.tile([1, 4], F32, tag=f"sti{tag}")
                    ago = dr.tile([NCORES, 4], F32, tag=f"sto{tag}",
                                  addr_space="Shared")
                    nc.sync.dma_start(agi, loc)
                    nc.gpsimd.collective_compute(
                        "AllGather", mybir.AluOpType.bypass, replica_groups=rg_all,
                        ins=[agi.opt()], outs=[ago.opt()])
                    gsb = st.tile([NCORES, 4], F32, tag=f"gsb{tag}", name=f"gsb{tag}")
                    nc.sync.dma_start(gsb, ago[:, :])
                    gps = pss.tile([1, 4], F32, tag="small")
                    nc.tensor.matmul(gps[:1], ones[:NCORES], gsb,
                                     start=True, stop=True)
                    mrow = per.tile([1, 4], F32, tag=f"mrow{tag}", name=f"mrow{tag}")
                    nc.scalar.mul(mrow, gps, 1.0 / CNT)
                m_ = mrow[0:1, 0:2]
                msq = st.tile([1, 2], F32, tag=f"msq{tag}", name=f"msq{tag}")
                nc.vector.tensor_mul(msq, m_, m_)
                var = per.tile([1, 2], F32, tag=f"var{tag}", name=f"var{tag}")
                nc.vector.tensor_sub(var, mrow[0:1, 2:4], msq)
                return m_, var

            m1, var1 = stats_round(y1, "r1")
            std1 = st.tile([1, 2], F32, tag="std1")
            nc.scalar.activation(std1, var1, AF.Sqrt, bias=epst[0:1, 0:1])
            rs1 = st.tile([1, 2], F32, tag="rs1")
            nc.vector.reciprocal(rs1, std1)
            nmrs1 = st.tile([1, 2], F32, tag="nmrs1")
            nc.vector.tensor_mul(nmrs1, m1, rs1)
            nc.scalar.mul(nmrs1, nmrs1, -1.0)
            pk1 = st.tile([1, 4], F32, tag="pk1")
            nc.vector.tensor_copy(pk1[:, 0:2], rs1)
            nc.vector.tensor_copy(pk1[:, 2:4], nmrs1)
            r1bc = pe_bcast(pk1[0:1, :], 4, "r1")
            # y1r = relu(y1 * rs + (-m*rs))  (in place, DVE)
            for bi, br in enumerate(("l", "g")):
                nc.vector.tensor_scalar(
                    y1[br], y1[br],
                    r1bc[:, bi:bi + 1], r1bc[:, 2 + bi:3 + bi],
                    op0=MUL, op1=ADD)
                nc.vector.tensor_scalar_max(y1[br], y1[br], 0.0)

            mr, vr = stats_round(y1, "r2")
            # per-channel alpha_l, alpha_g, beta  [1,4] each
            al = {}
            for bi, (br, coff) in enumerate((("l", 8), ("g", 12))):
                w2 = camb[0:1, coff:coff + 4]
                w2sq = st.tile([1, 4], F32, tag=f"w2sq{br}", name=f"w2sq{br}")
                nc.vector.tensor_mul(w2sq, w2, w2)
                nc.vector.tensor_scalar(w2sq, w2sq, vr[0:1, bi:bi + 1], EPS,
                                        op0=MUL, op1=ADD)
                nc.scalar.activation(w2sq, w2sq, AF.Sqrt)
                nc.vector.reciprocal(w2sq, w2sq)
                a_ = st.tile([1, 4], F32, tag=f"al{br}", name=f"al{br}")
                nc.vector.tensor_mul(a_, w2, w2sq)
                al[br] = a_
            beta = st.tile([1, 4], F32, tag="beta")
            bt = st.tile([1, 4], F32, tag="bt")
            nc.vector.tensor_scalar_mul(beta, al["l"], mr[0:1, 0:1])
            nc.vector.tensor_scalar_mul(bt, al["g"], mr[0:1, 1:2])
            nc.vector.tensor_add(beta, beta, bt)
            nc.scalar.mul(beta, beta, -1.0)
            pk2 = st.tile([1, 12], F32, tag="pk2")
            nc.vector.tensor_copy(pk2[:, 0:4], al["l"])
            nc.vector.tensor_copy(pk2[:, 4:8], al["g"])
            nc.vector.tensor_copy(pk2[:, 8:12], beta)
            r2bc = pe_bcast(pk2[0:1, :], 12, "r2")

            # fuse: acc = sum_c x_c * sigmoid(al_c*u + ag_c*w + beta_c)
            acc = per.tile([128, PW], F16, tag="acc", name="acc")
            zc = st.tile([128, PW], F16, tag="zc", bufs=1)
            zg = st.tile([128, PW], F16, tag="zg", bufs=1)
            for c in range(4):
                nc.vector.tensor_scalar(zc, y1["l"], r2bc[:, c:c + 1],
                                        r2bc[:, 8 + c:9 + c], op0=MUL, op1=ADD)
                nc.vector.tensor_scalar_mul(zg, y1["g"], r2bc[:, 4 + c:5 + c])
                nc.vector.tensor_add(zc, zc, zg)
                nc.scalar.activation(zc, zc, AF.Sigmoid)
                if c == 0:
                    nc.vector.tensor_mul(acc, xsp[CH[c]], zc)
                else:
                    nc.vector.tensor_mul(zg, xsp[CH[c]], zc)
                    nc.vector.tensor_add(acc, acc, zg)
            for s, (i0, isz) in enumerate(ISUBS):
                qrt = st.tile([128, 2], F32, tag="qrt", bufs=2)
                asl = acc[:, s * OH:(s + 1) * OH]
                nc.vector.tensor_mul(zc[:isz, 0:OH], asl[:isz], wabc[:isz, 0:OH])
                nc.vector.reduce_sum(qrt[:isz, 0:1], zc[:isz, 0:OH],
                                     axis=mybir.AxisListType.X)
                nc.vector.tensor_mul(zc[:isz, 0:OH], asl[:isz],
                                     wabc[:isz, OH:2 * OH])
                nc.vector.reduce_sum(qrt[:isz, 1:2], zc[:isz, 0:OH],
                                     axis=mybir.AxisListType.X)
                nc.sync.dma_start(qr_out[i0:i0 + isz, :], qrt[:isz])
    nc.compile()
    return nc


# ======================= host side ==================================

def _rearr(a, nk):
    # [KP, C] -> [128, NK*C] (block kt at cols [kt*C:(kt+1)*C])
    kp, c = a.shape
    return np.ascontiguousarray(
        a.reshape(nk, 128, c).transpose(1, 0, 2).reshape(128, nk * c))


def _prep(inputs):
    f16 = np.float16
    per_core = [dict() for _ in range(NCORES)]
    frows = [np.arange(a * CI, (a + 1) * CI) for a in range(NA)]
    valids = [(fr < NROWS) for fr in frows]
    fr_all = np.arange(MI)
    va_all = fr_all < NROWS

    for V in VIEWS:
        n, N, off, CJ, NK, KP, NJS = (V["name"], V["N"], V["off"], V["CJ"],
                                      V["NK"], V["KP"], V["NJS"])
        feat = np.asarray(inputs[f"feat_{n}"], np.float32)
        adj = np.asarray(inputs[f"adj_{n}"])
        W = np.asarray(inputs[f"W_{n}"], np.float64)
        a_src = np.asarray(inputs[f"a_src_{n}"], np.float64)
        a_dst = np.asarray(inputs[f"a_dst_{n}"], np.float64)
        M = (adj != 0)
        np.fill_diagonal(M, True)

        wsrc = W.T @ a_src
        wdst = W.T @ a_dst
        asrc = (feat.astype(np.float64) @ wsrc).astype(np.float32)   # [N]
        adst = (feat.astype(np.float64) @ wdst).astype(np.float32)   # [N]
        vrow = np.where(fr_all < OUT, fr_all, off + fr_all - OUT)
        vrow = np.where(va_all, vrow, 0)
        adstv = np.where(va_all, adst[vrow], 0.0).astype(np.float32)  # [MI]

        WTp = np.zeros((KP, 2 * OH), np.float32)
        WTp[:N, :OUT] = W.T
        featb = feat.astype(f16).astype(np.float32)

        featT_a, maskb_a = [], []
        for a in range(NA):
            j0, j1 = a * CJ, min((a + 1) * CJ, N)
            ft = np.zeros((KP, CJ), np.float32)
            ft[:N, :j1 - j0] = featb[j0:j1].T
            featT_a.append(_rearr(ft, NK).astype(f16))
            mb = np.full((NJS * 128, MI), MASKNEG, np.float32)
            msl = M[j0:j1][:, vrow]          # [CJa, MI] edge j->vrow(i)
            msl[:, ~va_all] = False
            mb[:j1 - j0, :] = (np.where(msl, 0.0, MASKNEG)
                               + asrc[j0:j1, None])
            maskb_a.append(mb.astype(f16))

        bpad = np.zeros((2 * OH,), np.float32)
        bpad[:OUT] = np.asarray(inputs[f"b_{n}"], np.float32)
        for c in range(NCORES):
            a, b = c % NA, c // NA
            Wx = WTp[:, b * OH:(b + 1) * OH]
            per_core[c][f"Wx_{n}"] = _rearr(Wx, NK).astype(f16)
            per_core[c][f"featT_{n}"] = featT_a[a]
            per_core[c][f"maskb_{n}"] = maskb_a[a]
            per_core[c][f"adst_{n}"] = adstv.reshape(1, MI).astype(f16)
            per_core[c][f"b_{n}"] = bpad[b * OH:(b + 1) * OH].reshape(1, OH).astype(f16)

    # collapsed pair-MLP vector + constant
    mW1 = np.asarray(inputs["mW1"], np.float64)
    mW2 = np.asarray(inputs["mW2"], np.float64)
    mW3 = np.asarray(inputs["mW3"], np.float64)
    mW4 = np.asarray(inputs["mW4"], np.float64)
    w432 = mW4 @ mW3 @ mW2
    wfull = (w432 @ mW1)[0]
    cconst = (np.asarray(inputs["mb1"], np.float64) @ w432[0]
              + np.asarray(inputs["mb2"], np.float64) @ (mW4 @ mW3)[0]
              + np.asarray(inputs["mb3"], np.float64) @ mW4[0]
              + np.asarray(inputs["mb4"], np.float64)[0])
    wap = np.zeros((2 * OH,), np.float64)
    wap[:OUT] = wfull[:OUT] / 4.0
    wbp = np.zeros((2 * OH,), np.float64)
    wbp[:OUT] = wfull[OUT:] / 4.0

    camw = np.concatenate([
        np.asarray(inputs["lw1"], np.float32).ravel(),
        np.asarray(inputs["gw1"], np.float32).ravel(),
        np.asarray(inputs["lw2"], np.float32).ravel(),
        np.asarray(inputs["gw2"], np.float32).ravel()]).reshape(1, 16)

    md = np.asarray(inputs["mirna_disease"], np.float32)
    mdp = np.zeros((MI, 2 * OH), np.float32)
    mdp[:NROWS, :OUT] = md
    for c in range(NCORES):
        a, b = c % NA, c // NA
        per_core[c]["md"] = mdp[a * CI:(a + 1) * CI, b * OH:(b + 1) * OH].astype(f16)
        per_core[c]["validi"] = valids[a].astype(np.float32).reshape(CI, 1)
        per_core[c]["camw"] = camw
        per_core[c]["wab"] = np.stack(
            [wap[b * OH:(b + 1) * OH], wbp[b * OH:(b + 1) * OH]]).astype(f16)
        ncols = OH if b == 0 else OUT - OH
        nrowsv = int(valids[a].sum())
        per_core[c]["cntinv"] = np.full((1, 1), 1.0 / (nrowsv * ncols), np.float32)
    return per_core, float(cconst)


def kernel(**inputs):
    global LAST_RESULTS
    if "nc" not in _CACHE:
        _CACHE["nc"] = build_graph()
    nc = _CACHE["nc"]
    in_maps, cconst = _prep(inputs)
    res = run_bass_kernel_spmd(nc, in_maps, core_ids=list(range(NCORES)))
    LAST_RESULTS = res
    qr_halves = [np.concatenate([np.asarray(res.results[b * NA + a]["qr"])
                                 for a in range(NA)]) for b in range(2)]
    qr = qr_halves[0] + qr_halves[1]
    q, r = qr[:NROWS, 0], qr[:NROWS, 1]
    ts = np.asarray(inputs["test_sample"])
    out = (q[ts[:, 0]] + r[ts[:, 1]] + cconst).astype(np.float32)
    return out.reshape(NPAIRS, 1)


# revision 28
# speedup vs baseline: 2.2285x; 2.2285x over previous
"""Trainium2 Bass kernel for the MDA GNN (3x GAT views + MS-CAM fusion + pair MLP).

Distribution over 8 NeuronCores, core c = (a, b): a = c % 4 (row quarter),
b = c // 4 (output-feature half; 901 -> 2x452).

Design (vs the AllGather baseline):
  - attention logits asrc/adst are host-precomputed matvecs, shipped as tiny
    inputs: no featU on device, no logit columns in the collective payload;
    asrc[j] is folded into the mask tensor (mask = asrc + {0 | -100})
  - softmax weights pt[j, i] for ALL 1784 fused rows i depend only on
    inputs, so DVE/Act compute them from t=0, overlapped with stage 1:
    em = adst + mask (2x DVE), lrelu via (0.2*em) max em (4x + 2x DVE),
    exp on Act (fp16 out; masked entries underflow fp16 to ~0)
  - stage 1: h[j in quarter a, half b] = feat @ W[half].T in fp16,
    k-tile-outer over j-subtile groups of 4 so PE starts on the first
    featT chunk; fp16 h tiles stay in SBUF
  - stage 2: local-j partial attention sums [1784, 453] (452 cols + rowsum
    via an ones column) in fp16; all three views packed into ONE fp16
    ReduceScatter over each 4-core half-group -- replaces the three big
    AllGathers and the agout round-trips entirely
  - epilogue v = relu(num/rowsum + b), MS-CAM in fp16 on DVE fast modes,
    BN stats per-core (LOCAL_STATS) or via two tiny AllGathers
  - output per-core q,r partials; host sums halves and applies the
    collapsed pair MLP
"""

import numpy as np
import ml_dtypes

import concourse.bass as bass
import concourse.mybir as mybir
import concourse.tile as tile
from concourse import bacc
from concourse.bass_utils import run_bass_kernel_spmd

F16 = mybir.dt.float16
F32 = mybir.dt.float32
AF = mybir.ActivationFunctionType
MUL = mybir.AluOpType.mult
ADD = mybir.AluOpType.add
MAX = mybir.AluOpType.max

NCORES = 8
NA = 4
OUT = 901
OH = 452              # half width (904 = 2*452)
OHR = OH + 1          # + rowsum column
NROWS = 1778
MI = 1784
CI = MI // NA         # 446 rows per core after scatter
NIT = -(-MI // 128)   # stage-2 i tiles (14, last = 120)
NPAIRS = 4096
EPS = 1e-5
CNT = float(NROWS * OUT)
MASKNEG = -100.0
LOCAL_STATS = True   # per-core BN stats (approx) instead of global AGs

VIEWS = [
    dict(name="mrna", N=3929, off=3052),
    dict(name="inc", N=2459, off=1582),
    dict(name="drug", N=2060, off=1183),
]
for V in VIEWS:
    V["CJ"] = -(-V["N"] // NA)               # per-core source rows
    V["NK"] = -(-V["N"] // 128)
    V["KP"] = V["NK"] * 128
    V["NJS"] = -(-V["CJ"] // 128)            # j subtiles

ISUBS = [(0, 128), (128, 128), (256, 128), (384, CI - 384)]

_CACHE = {}
LAST_RESULTS = None


def _bcast(ap, parts, cols, offset=0):
    """Partition-broadcast AP over a DRAM row."""
    return bass.AP(tensor=ap.tensor, offset=ap.offset + offset,
                   ap=[[0, parts], [1, cols]])


def build_graph():
    nc = bacc.Bacc("TRN2", target_bir_lowering=False, debug=False,
                   enable_asserts=False, num_devices=NCORES)
    ins = {}
    for V in VIEWS:
        n = V["name"]
        ins[f"featT_{n}"] = nc.dram_tensor(
            f"featT_{n}", [128, V["NK"] * V["CJ"]], F16, kind="ExternalInput").ap()
        ins[f"Wx_{n}"] = nc.dram_tensor(
            f"Wx_{n}", [128, V["NK"] * OH], F16, kind="ExternalInput").ap()
        ins[f"maskb_{n}"] = nc.dram_tensor(
            f"maskb_{n}", [V["NJS"] * 128, MI], F16, kind="ExternalInput").ap()
        ins[f"adst_{n}"] = nc.dram_tensor(
            f"adst_{n}", [1, MI], F16, kind="ExternalInput").ap()
        ins[f"b_{n}"] = nc.dram_tensor(
            f"b_{n}", [1, OH], F16, kind="ExternalInput").ap()
    ins["md"] = nc.dram_tensor("md", [CI, OH], F16, kind="ExternalInput").ap()
    ins["validi"] = nc.dram_tensor("validi", [CI, 1], F32, kind="ExternalInput").ap()
    ins["camw"] = nc.dram_tensor("camw", [1, 16], F32, kind="ExternalInput").ap()
    ins["wab"] = nc.dram_tensor("wab", [2, OH], F16, kind="ExternalInput").ap()
    ins["cntinv"] = nc.dram_tensor("cntinv", [1, 1], F32, kind="ExternalInput").ap()
    qr_out = nc.dram_tensor("qr", [CI, 2], F32, kind="ExternalOutput").ap()
    rg_half = [[0, 1, 2, 3], [4, 5, 6, 7]]
    rg_all = [list(range(NCORES))]
    NV = len(VIEWS)
    FTMAX = max(V["NK"] * V["CJ"] for V in VIEWS)
    WXMAX = max(V["NK"] * OH for V in VIEWS)

    with tile.TileContext(nc) as tc:
        with (
            tc.tile_pool(name="persist", bufs=1) as per,
            tc.tile_pool(name="stream", bufs=2) as st,
            tc.tile_pool(name="dram", bufs=1, space="DRAM") as dr,
            tc.tile_pool(name="ps_s1", bufs=1, space="PSUM") as ps1,
            tc.tile_pool(name="ps_s2", bufs=2, space="PSUM") as ps2p,
            tc.tile_pool(name="ps_sm", bufs=1, space="PSUM") as pss,
        ):
            # ---- non-DMA constants ----
            ones = per.tile([128, 1], F32, tag="ones")
            nc.vector.memset(ones, 1.0)
            onesrow = per.tile([1, 128], F32, tag="onesrow")
            nc.vector.memset(onesrow, 1.0)
            epst = per.tile([1, 1], F32, tag="epst")
            nc.vector.memset(epst, EPS)
            camb = per.tile([128, 16], F32, tag="camb")
            cnti = per.tile([1, 1], F32, tag="cnti")
            valid, invalid, mdt = {}, {}, {}
            adstbc, bbc = {}, {}

            def late_dmas():
                # everything not needed until the epilogue/CAM tail
                nc.sync.dma_start(camb, _bcast(ins["camw"], 128, 16))
                nc.sync.dma_start(cnti, ins["cntinv"][:, :])
                for s, (i0, isz) in enumerate(ISUBS):
                    valid[s] = per.tile([128, 1], F32, tag=f"valid{s}",
                                        name=f"valid{s}")
                    nc.sync.dma_start(valid[s][:isz], ins["validi"][i0:i0 + isz, :])
                    invalid[s] = per.tile([128, 1], F32, tag=f"invalid{s}",
                                          name=f"invalid{s}")
                    nc.vector.tensor_scalar(invalid[s][:isz], valid[s][:isz],
                                            -1.0, 1.0, op0=MUL, op1=ADD)
                wabc = per.tile([128, 2 * OH], F16, tag="wabc")
                nc.sync.dma_start(wabc[:, 0:OH], _bcast(ins["wab"], 128, OH, offset=0))
                nc.sync.dma_start(wabc[:, OH:2 * OH],
                                  _bcast(ins["wab"], 128, OH, offset=OH))
                for vi, V in enumerate(VIEWS):
                    n = V["name"]
                    t = per.tile([128, OH], F16, tag=f"bbc{vi}", name=f"bbc{vi}")
                    nc.sync.dma_start(t, _bcast(ins[f"b_{n}"], 128, OH))
                    bbc[vi] = t
                return wabc

            # single featT/Wx buffers, reused across views (WAR-tracked)
            ftbuf = per.tile([128, FTMAX], F16, tag="ftbuf")
            wxbuf = per.tile([128, WXMAX], F16, tag="wxbuf")
            mbt = {}

            rsin = {vi: dr.tile([MI, OHR], F16, tag=f"rsin{vi}", name=f"rsin{vi}")
                    for vi in range(NV)}
            rsout = {vi: dr.tile([CI, OHR], F16, tag=f"rsout{vi}", name=f"rsout{vi}")
                     for vi in range(NV)}

            hsub, pts = {}, {}
            for vi, V in enumerate(VIEWS):
                n, CJ, NK, NJS = V["name"], V["CJ"], V["NK"], V["NJS"]
                # -------- input DMAs: first weight chunks lead the queue ----
                t = per.tile([128, MI], F16, tag=f"adstbc{vi}", name=f"adstbc{vi}")
                adstbc[vi] = t
                nch = 4
                ktc = [(NK * c // nch, NK * (c + 1) // nch) for c in range(nch)]
                for ci, (k0, k1) in enumerate(ktc):
                    nc.sync.dma_start(wxbuf[:, k0 * OH:k1 * OH],
                                      ins[f"Wx_{n}"][:, k0 * OH:k1 * OH])
                    nc.sync.dma_start(ftbuf[:, k0 * CJ:k1 * CJ],
                                      ins[f"featT_{n}"][:, k0 * CJ:k1 * CJ])
                    if ci == 0:
                        nc.sync.dma_start(t, _bcast(ins[f"adst_{n}"], 128, MI))
                        for s in range(min(2, NJS)):
                            mb = st.tile([128, MI], F16, tag="mb", bufs=5,
                                         name=f"mb{vi}_{s}")
                            nc.sync.dma_start(
                                mb, ins[f"maskb_{n}"][s * 128:(s + 1) * 128, :])
                            mbt[(vi, s)] = mb
                for s in range(2, NJS):
                    mb = st.tile([128, MI], F16, tag="mb", bufs=5,
                                 name=f"mb{vi}_{s}")
                    nc.sync.dma_start(mb, ins[f"maskb_{n}"][s * 128:(s + 1) * 128, :])
                    mbt[(vi, s)] = mb
                if vi == 0:
                    wabc = late_dmas()

                # -------- stage 1: kt-outer over j-subtile groups of 4 -----
                for s in range(NJS):
                    hsub[(vi, s)] = per.tile([128, OHR], F16, tag=f"h{s}",
                                             name=f"h{vi}_{s}")
                groups = [list(range(g, min(g + 4, NJS))) for g in range(0, NJS, 4)]
                for grp in groups:
                    hp = {s: ps1.tile([128, OH], F32, tag=f"s1ps{gi}",
                                      name=f"s1ps{vi}_{s}")
                          for gi, s in enumerate(grp)}
                    for kt in range(NK):
                        for s in grp:
                            pj = min(128, CJ - s * 128)
                            nc.tensor.matmul(
                                hp[s][:pj],
                                ftbuf[:, kt * CJ + s * 128: kt * CJ + s * 128 + pj],
                                wxbuf[:, kt * OH:(kt + 1) * OH],
                                start=(kt == 0), stop=(kt == NK - 1))
                    for s in grp:
                        pj = min(128, CJ - s * 128)
                        ht = hsub[(vi, s)]
                        if pj < 128:
                            nc.vector.memset(ht, 0.0)
                        nc.scalar.copy(ht[:pj, 0:OH], hp[s][:pj])
                        nc.vector.memset(ht[:pj, OH:OHR], 1.0)

                # -------- softmax weights (DVE/Act, input-only deps) -------
                for s in range(NJS):
                    pts[(vi, s)] = per.tile([128, MI], F16, tag=f"pt{s}",
                                            name=f"pt{vi}_{s}")
                    em = st.tile([128, MI], F16, tag="em", bufs=2)
                    pt = pts[(vi, s)]
                    nc.vector.tensor_add(em, adstbc[vi], mbt[(vi, s)])
                    nc.vector.tensor_scalar_mul(pt, em, 0.2)
                    nc.vector.tensor_max(em, em, pt)
                    nc.scalar.activation(pt, em, AF.Exp)

                # -------- stage 2: partial sums over local j ---------------
                for it in range(NIT):
                    isz = min(128, MI - it * 128)
                    i0 = it * 128
                    pp = ps2p.tile([128, OHR], F32, tag="s2ps")
                    for s in range(NJS):
                        nc.tensor.matmul(
                            pp[:isz], pts[(vi, s)][:, i0:i0 + isz],
                            hsub[(vi, s)][:, :],
                            start=(s == 0), stop=(s == NJS - 1))
                    ct = st.tile([128, OHR], F16, tag="ct", bufs=3)
                    if it % 2 == 0:
                        nc.vector.tensor_copy(ct[:isz], pp[:isz])
                    else:
                        nc.scalar.copy(ct[:isz], pp[:isz])
                    nc.sync.dma_start(rsin[vi][i0:i0 + isz, :], ct[:isz])

                nc.gpsimd.collective_compute(
                    "ReduceScatter", ADD, replica_groups=rg_half,
                    ins=[rsin[vi].opt()], outs=[rsout[vi].opt()])

            # =================== epilogue + CAM (packed) ===============
            # xsp[vi][:, s*OH:(s+1)*OH] = x for subtile s; xsp[3] = mirna_disease
            # channel attention channel c -> (drug, inc, mrna, md) = view
            # indices (2, 1, 0, md); cmap[vi] = channel of view vi
            PW = len(ISUBS) * OH
            CH = [2, 1, 0, 3]
            CMAP = [2, 1, 0]
            xsp = {c: per.tile([128, PW], F16, tag=f"xsp{c}", name=f"xsp{c}")
                   for c in range(4)}
            for c in range(4):
                nc.vector.memset(xsp[c], 0.0)
            for s, (i0, isz) in enumerate(ISUBS):
                nc.sync.dma_start(xsp[3][:isz, s * OH:(s + 1) * OH],
                                  ins["md"][i0:i0 + isz, :])
            # y1 initialized with the input-only md term (channel 3)
            y1 = {}
            tm = st.tile([128, PW], F16, tag="y1tmp", bufs=1)
            for br, coff in (("l", 0), ("g", 4)):
                t = per.tile([128, PW], F16, tag=f"y1{br}", name=f"y1{br}")
                nc.vector.tensor_scalar_mul(t, xsp[3], camb[:, coff + 3:coff + 4])
                y1[br] = t
            for vi in range(NV):
                for s, (i0, isz) in enumerate(ISUBS):
                    rsg = st.tile([128, OHR], F16, tag="rsg", bufs=2)
                    nc.sync.dma_start(rsg[:isz], rsout[vi][i0:i0 + isz, :])
                    rsum = st.tile([128, 1], F32, tag="rsum")
                    nc.vector.tensor_add(rsum[:isz], rsg[:isz, OH:OHR],
                                         invalid[s][:isz])
                    rs = st.tile([128, 1], F32, tag="rs")
                    nc.vector.reciprocal(rs[:isz], rsum[:isz])
                    xv = xsp[vi][:, s * OH:(s + 1) * OH]
                    nc.vector.tensor_scalar_mul(xv[:isz], rsg[:isz, 0:OH], rs[:isz])
                    nc.vector.tensor_add(xv[:isz], xv[:isz], bbc[vi][:isz])
                    nc.vector.tensor_scalar_max(xv[:isz], xv[:isz], 0.0)
                for br, coff in (("l", 0), ("g", 4)):
                    cc = coff + CMAP[vi]
                    nc.vector.tensor_scalar_mul(tm, xsp[vi], camb[:, cc:cc + 1])
                    nc.vector.tensor_add(y1[br], y1[br], tm)

            def pe_bcast(src_row, cols, tag):
                """Broadcast [1, cols] SBUF row to [128, cols] via PE."""
                bps = pss.tile([128, 16], F32, tag="bps", name=f"bps{tag}")[:, 0:cols]
                nc.tensor.matmul(bps, onesrow[0:1, :], src_row,
                                 start=True, stop=True)
                out = per.tile([128, cols], F32, tag=f"bc{tag}", name=f"bc{tag}")
                nc.vector.tensor_copy(out, bps)
                return out

            def stats_round(srcs, tag):
                # per-core sums over valid rows: cols (S_l, S_g, Q_l, Q_g)
                stp = pss.tile([1, 4], F32, tag="small")
                nsub = len(ISUBS)
                for s, (i0, isz) in enumerate(ISUBS):
                    sc = st.tile([128, 4], F32, tag="scst", bufs=2)
                    sq = st.tile([128, OH], F32, tag="sqscr", bufs=1)
                    for bi, br in enumerate(("l", "g")):
                        ysl = srcs[br][:, s * OH:(s + 1) * OH]
                        nc.vector.reduce_sum(sc[:isz, bi:bi + 1], ysl[:isz],
                                             axis=mybir.AxisListType.X)
                        nc.scalar.activation(sq[:isz], ysl[:isz],
                                             AF.Square,
                                             accum_out=sc[:isz, 2 + bi:3 + bi])
                    nc.vector.tensor_scalar_mul(sc[:isz], sc[:isz], valid[s][:isz])
                    nc.tensor.matmul(stp[:1], ones[:isz], sc[:isz],
                                     start=(s == 0), stop=(s == nsub - 1))
                if LOCAL_STATS:
                    gsb = st.tile([1, 4], F32, tag=f"loc{tag}", name=f"loc{tag}")
                    nc.vector.tensor_copy(gsb, stp)
                    mrow = per.tile([1, 4], F32, tag=f"mrow{tag}", name=f"mrow{tag}")
                    nc.vector.tensor_scalar_mul(mrow, gsb, cnti[0:1, 0:1])
                else:
                    loc = st.tile([1, 4], F32, tag=f"loc{tag}", name=f"loc{tag}")
                    nc.vector.tensor_copy(loc, stp)
                    agi = dr.tile([1, 4], F32, tag=f"sti{tag}")
                    ago = dr.tile([NCORES, 4], F32, tag=f"sto{tag}",
                                  addr_space="Shared")
                    nc.sync.dma_start(agi, loc)
                    nc.gpsimd.collective_compute(
                        "AllGather", mybir.AluOpType.bypass, replica_groups=rg_all,
                        ins=[agi.opt()], outs=[ago.opt()])
                    gsb = st.tile([NCORES, 4], F32, tag=f"gsb{tag}", name=f"gsb{tag}")
                    nc.sync.dma_start(gsb, ago[:, :])
                    gps = pss.tile([1, 4], F32, tag="small")
                    nc.tensor.matmul(gps[:1], ones[:NCORES], gsb,
                                     start=True, stop=True)
                    mrow = per.tile([1, 4], F32, tag=f"mrow{tag}", name=f"mrow{tag}")
                    nc.scalar.mul(mrow, gps, 1.0 / CNT)
                m_ = mrow[0:1, 0:2]
                msq = st.tile([1, 2], F32, tag=f"msq{tag}", name=f"msq{tag}")
                nc.vector.tensor_mul(msq, m_, m_)
                var = per.tile([1, 2], F32, tag=f"var{tag}", name=f"var{tag}")
                nc.vector.tensor_sub(var, mrow[0:1, 2:4], msq)
                return m_, var

            m1, var1 = stats_round(y1, "r1")
            std1 = st.tile([1, 2], F32, tag="std1")
            nc.scalar.activation(std1, var1, AF.Sqrt, bias=epst[0:1, 0:1])
            rs1 = st.tile([1, 2], F32, tag="rs1")
            nc.vector.reciprocal(rs1, std1)
            nmrs1 = st.tile([1, 2], F32, tag="nmrs1")
            nc.vector.tensor_mul(nmrs1, m1, rs1)
            nc.scalar.mul(nmrs1, nmrs1, -1.0)
            pk1 = st.tile([1, 4], F32, tag="pk1")
            nc.vector.tensor_copy(pk1[:, 0:2], rs1)
            nc.vector.tensor_copy(pk1[:, 2:4], nmrs1)
            r1bc = pe_bcast(pk1[0:1, :], 4, "r1")
            # y1r = relu(y1 * rs + (-m*rs))  (in place, DVE)
            for bi, br in enumerate(("l", "g")):
                nc.vector.tensor_scalar(
                    y1[br], y1[br],
                    r1bc[:, bi:bi + 1], r1bc[:, 2 + bi:3 + bi],
                    op0=MUL, op1=ADD)
                nc.vector.tensor_scalar_max(y1[br], y1[br], 0.0)

            mr, vr = stats_round(y1, "r2")
            # per-channel alpha_l, alpha_g, beta  [1,4] each
            al = {}
            for bi, (br, coff) in enumerate((("l", 8), ("g", 12))):
                w2 = camb[0:1, coff:coff + 4]
                w2sq = st.tile([1, 4], F32, tag=f"w2sq{br}", name=f"w2sq{br}")
                nc.vector.tensor_mul(w2sq, w2, w2)
                nc.vector.tensor_scalar(w2sq, w2sq, vr[0:1, bi:bi + 1], EPS,
                                        op0=MUL, op1=ADD)
                nc.scalar.activation(w2sq, w2sq, AF.Sqrt)
                nc.vector.reciprocal(w2sq, w2sq)
                a_ = st.tile([1, 4], F32, tag=f"al{br}", name=f"al{br}")
                nc.vector.tensor_mul(a_, w2, w2sq)
                al[br] = a_
            beta = st.tile([1, 4], F32, tag="beta")
            bt = st.tile([1, 4], F32, tag="bt")
            nc.vector.tensor_scalar_mul(beta, al["l"], mr[0:1, 0:1])
            nc.vector.tensor_scalar_mul(bt, al["g"], mr[0:1, 1:2])
            nc.vector.tensor_add(beta, beta, bt)
            nc.scalar.mul(beta, beta, -1.0)
            pk2 = st.tile([1, 12], F32, tag="pk2")
            nc.vector.tensor_copy(pk2[:, 0:4], al["l"])
            nc.vector.tensor_copy(pk2[:, 4:8], al["g"])
            nc.vector.tensor_copy(pk2[:, 8:12], beta)
            r2bc = pe_bcast(pk2[0:1, :], 12, "r2")

            # fuse: acc = sum_c x_c * sigmoid(al_c*u + ag_c*w + beta_c)
            acc = per.tile([128, PW], F16, tag="acc", name="acc")
            zc = st.tile([128, PW], F16, tag="zc", bufs=1)
            zg = st.tile([128, PW], F16, tag="zg", bufs=1)
            for c in range(4):
                nc.vector.tensor_scalar(zc, y1["l"], r2bc[:, c:c + 1],
                                        r2bc[:, 8 + c:9 + c], op0=MUL, op1=ADD)
                nc.vector.tensor_scalar_mul(zg, y1["g"], r2bc[:, 4 + c:5 + c])
                nc.vector.tensor_add(zc, zc, zg)
                nc.scalar.activation(zc, zc, AF.Sigmoid)
                if c == 0:
                    nc.vector.tensor_mul(acc, xsp[CH[c]], zc)
                else:
                    nc.vector.tensor_mul(zg, xsp[CH[c]], zc)
                    nc.vector.tensor_add(acc, acc, zg)
            for s, (i0, isz) in enumerate(ISUBS):
                qrt = st.tile([128, 2], F32, tag="qrt", bufs=2)
                asl = acc[:, s * OH:(s + 1) * OH]
                nc.vector.tensor_mul(zc[:isz, 0:OH], asl[:isz], wabc[:isz, 0:OH])
                nc.vector.reduce_sum(qrt[:isz, 0:1], zc[:isz, 0:OH],
                                     axis=mybir.AxisListType.X)
                nc.vector.tensor_mul(zc[:isz, 0:OH], asl[:isz],
                                     wabc[:isz, OH:2 * OH])
                nc.vector.reduce_sum(qrt[:isz, 1:2], zc[:isz, 0:OH],
                                     axis=mybir.AxisListType.X)
                nc.sync.dma_start(qr_out[i0:i0 + isz, :], qrt[:isz])
    nc.compile()
    return nc


# ======================= host side ==================================

def _rearr(a, nk):
    # [KP, C] -> [128, NK*C] (block kt at cols [kt*C:(kt+1)*C])
    kp, c = a.shape
    return np.ascontiguousarray(
        a.reshape(nk, 128, c).transpose(1, 0, 2).reshape(128, nk * c))


def _prep(inputs):
    f16 = np.float16
    per_core = [dict() for _ in range(NCORES)]
    frows = [np.arange(a * CI, (a + 1) * CI) for a in range(NA)]
    valids = [(fr < NROWS) for fr in frows]
    fr_all = np.arange(MI)
    va_all = fr_all < NROWS

    for V in VIEWS:
        n, N, off, CJ, NK, KP, NJS = (V["name"], V["N"], V["off"], V["CJ"],
                                      V["NK"], V["KP"], V["NJS"])
        feat = np.asarray(inputs[f"feat_{n}"], np.float32)
        adj = np.asarray(inputs[f"adj_{n}"])
        W = np.asarray(inputs[f"W_{n}"], np.float64)
        a_src = np.asarray(inputs[f"a_src_{n}"], np.float64)
        a_dst = np.asarray(inputs[f"a_dst_{n}"], np.float64)
        M = (adj != 0)
        np.fill_diagonal(M, True)

        wsrc = W.T @ a_src
        wdst = W.T @ a_dst
        asrc = (feat.astype(np.float64) @ wsrc).astype(np.float32)   # [N]
        adst = (feat.astype(np.float64) @ wdst).astype(np.float32)   # [N]
        vrow = np.where(fr_all < OUT, fr_all, off + fr_all - OUT)
        vrow = np.where(va_all, vrow, 0)
        adstv = np.where(va_all, adst[vrow], 0.0).astype(np.float32)  # [MI]

        WTp = np.zeros((KP, 2 * OH), np.float32)
        WTp[:N, :OUT] = W.T
        featb = feat.astype(f16).astype(np.float32)

        featT_a, maskb_a = [], []
        for a in range(NA):
            j0, j1 = a * CJ, min((a + 1) * CJ, N)
            ft = np.zeros((KP, CJ), np.float32)
            ft[:N, :j1 - j0] = featb[j0:j1].T
            featT_a.append(_rearr(ft, NK).astype(f16))
            mb = np.full((NJS * 128, MI), MASKNEG, np.float32)
            msl = M[j0:j1][:, vrow]          # [CJa, MI] edge j->vrow(i)
            msl[:, ~va_all] = False
            mb[:j1 - j0, :] = (np.where(msl, 0.0, MASKNEG)
                               + asrc[j0:j1, None])
            maskb_a.append(mb.astype(f16))

        bpad = np.zeros((2 * OH,), np.float32)
        bpad[:OUT] = np.asarray(inputs[f"b_{n}"], np.float32)
        for c in range(NCORES):
            a, b = c % NA, c // NA
            Wx = WTp[:, b * OH:(b + 1) * OH]
            per_core[c][f"Wx_{n}"] = _rearr(Wx, NK).astype(f16)
            per_core[c][f"featT_{n}"] = featT_a[a]
            per_core[c][f"maskb_{n}"] = maskb_a[a]
            per_core[c][f"adst_{n}"] = adstv.reshape(1, MI).astype(f16)
            per_core[c][f"b_{n}"] = bpad[b * OH:(b + 1) * OH].reshape(1, OH).astype(f16)

    # collapsed pair-MLP vector + constant
    mW1 = np.asarray(inputs["mW1"], np.float64)
    mW2 = np.asarray(inputs["mW2"], np.float64)
    mW3 = np.asarray(inputs["mW3"], np.float64)
    mW4 = np.asarray(inputs["mW4"], np.float64)
    w432 = mW4 @ mW3 @ mW2
    wfull = (w432 @ mW1)[0]
    cconst = (np.asarray(inputs["mb1"], np.float64) @ w432[0]
              + np.asarray(inputs["mb2"], np.float64) @ (mW4 @ mW3)[0]
              + np.asarray(inputs["mb3"], np.float64) @ mW4[0]
              + np.asarray(inputs["mb4"], np.float64)[0])
    wap = np.zeros((2 * OH,), np.float64)
    wap[:OUT] = wfull[:OUT] / 4.0
    wbp = np.zeros((2 * OH,), np.float64)
    wbp[:OUT] = wfull[OUT:] / 4.0

    camw = np.concatenate([
        np.asarray(inputs["lw1"], np.float32).ravel(),
        np.asarray(inputs["gw1"], np.float32).ravel(),
        np.asarray(inputs["lw2"], np.float32).ravel(),
        np.asarray(inputs["gw2"], np.float32).ravel()]).reshape(1, 16)

    md = np.asarray(inputs["mirna_disease"], np.float32)
    mdp = np.zeros((MI, 2 * OH), np.float32)
    mdp[:NROWS, :OUT] = md
    for c in range(NCORES):
        a, b = c % NA, c // NA
        per_core[c]["md"] = mdp[a * CI:(a + 1) * CI, b * OH:(b + 1) * OH].astype(f16)
        per_core[c]["validi"] = valids[a].astype(np.float32).reshape(CI, 1)
        per_core[c]["camw"] = camw
        per_core[c]["wab"] = np.stack(
            [wap[b * OH:(b + 1) * OH], wbp[b * OH:(b + 1) * OH]]).astype(f16)
        ncols = OH if b == 0 else OUT - OH
        nrowsv = int(valids[a].sum())
        per_core[c]["cntinv"] = np.full((1, 1), 1.0 / (nrowsv * ncols), np.float32)
    return per_core, float(cconst)


def kernel(**inputs):
    global LAST_RESULTS
    if "nc" not in _CACHE:
        _CACHE["nc"] = build_graph()
    nc = _CACHE["nc"]
    in_maps, cconst = _prep(inputs)
    res = run_bass_kernel_spmd(nc, in_maps, core_ids=list(range(NCORES)))
    LAST_RESULTS = res
    qr_halves = [np.concatenate([np.asarray(res.results[b * NA + a]["qr"])
                                 for a in range(NA)]) for b in range(2)]
    qr = qr_halves[0] + qr_halves[1]
    q, r = qr[:NROWS, 0], qr[:NROWS, 1]
    ts = np.asarray(inputs["test_sample"])
    out = (q[ts[:, 0]] + r[ts[:, 1]] + cconst).astype(np.float32)
    return out.reshape(NPAIRS, 1)
